# revision 1
# baseline (speedup 1.0000x reference)
"""LinearAttention (relu feature map) + residual + LayerNorm on 8 TRN2 cores.

Reference (per batch b):
  q = relu(x @ Wq.T + bq); k = relu(x @ Wk.T + bk); v = x @ Wv.T + bv
  kv[h] = sum_n k[n,h,:] outer v[n,h,:];  k_sum[h] = sum_n k[n,h,:]
  denom = max(q . k_sum, 1e-6); ctx = q @ kv
  y = ctx/denom + x; out = LayerNorm(y) * gamma + beta

Sharding: core c handles (b = c//2, token half = c%2) -> T=2048 tokens.
kv/k_sum are partial sums over the core's tokens; a pairwise AllReduce
([0,1],[2,3],...) merges them. Everything else is core-local.

Matmuls run as float32r (TF32-like, full PE rate at free dim >= 256).
Projections contract input channels; x is supplied transposed (xt) so
its [128,128] chunks serve as stationary operands for k/v (normal
layout out) and as moving operands for q (transposed layout out, which
ctx/denom need since they contract channels).
"""
import numpy as np

import concourse.bass as bass
import concourse.tile as tile
from concourse import bacc, mybir
from concourse.bass_utils import run_bass_kernel_spmd
from concourse.bass import ts

B, NTOK, DIM, H, HD = 4, 4096, 1024, 16, 64
T = 2048          # tokens per core
P = 128           # partitions
KC = DIM // P     # 8 channel chunks
NPAIR = KC        # 8 head pairs (one per 128-channel chunk)
TT1 = T // P      # 16 token tiles in phase 1
F2 = 512          # phase-2 token tile (free dim)
TT2 = T // F2     # 4 phase-2 tiles
EPS_DENOM = 1e-6
EPS_LN = 1e-5
N_CORES = 8

F32 = mybir.dt.float32
F32R = mybir.dt.float32r
AF = mybir.ActivationFunctionType
ALU = mybir.AluOpType


def build(trace_sim: bool = False, dbg: bool = False) -> "bacc.Bacc":
    nc = bacc.Bacc("TRN2", target_bir_lowering=False, debug=False,
                   num_devices=N_CORES)

    xt_in = nc.dram_tensor("xt", [DIM, T], F32R, kind="ExternalInput").ap()
    xn_in = nc.dram_tensor("xn", [T, DIM], F32, kind="ExternalInput").ap()
    wqt_in = nc.dram_tensor("wqt", [DIM, DIM], F32R, kind="ExternalInput").ap()
    wkt_in = nc.dram_tensor("wkt", [DIM, DIM], F32R, kind="ExternalInput").ap()
    wvt_in = nc.dram_tensor("wvt", [DIM, DIM], F32R, kind="ExternalInput").ap()
    bq_in = nc.dram_tensor("bq", [DIM], F32, kind="ExternalInput").ap()
    bk_in = nc.dram_tensor("bk", [1, DIM], F32R, kind="ExternalInput").ap()
    bv_in = nc.dram_tensor("bv", [1, DIM], F32R, kind="ExternalInput").ap()
    gamma_in = nc.dram_tensor("gamma", [DIM], F32, kind="ExternalInput").ap()
    beta_in = nc.dram_tensor("beta", [DIM], F32, kind="ExternalInput").ap()
    yn_out = nc.dram_tensor("yn", [T, DIM], F32, kind="ExternalOutput").ap()
    if dbg:
        dbg_outs = {
            name: nc.dram_tensor(name, shape, F32, kind="ExternalOutput").ap()
            for name, shape in [
                ("dbg_k", [P, DIM]), ("dbg_v", [P, DIM]),
                ("dbg_kvred", [P, NPAIR * HD + KC]),
                ("dbg_q", [P, KC, F2]),
                ("dbg_ctx", [P, DIM]), ("dbg_den", [P, H]),
                ("dbg_y", [P, DIM]),
            ]
        }

    def bcast_dram_row(ap, n):
        # DRAM [D] -> [[0,n],[1,D]] so DMA replicates the row to n partitions
        return bass.AP(tensor=ap.tensor, offset=ap.offset,
                       ap=[[0, n]] + list(ap.ap))

    with tile.TileContext(nc, trace_sim=trace_sim) as tc:
        with (
            tc.tile_pool(name="persist", bufs=1) as persist,
            tc.tile_pool(name="dram", bufs=2, space="DRAM") as dram,
        ):
            xt_sb = persist.tile([P, KC, T], F32R)
            nc.sync.dma_start(xt_sb[:], xt_in.rearrange("(kc p) t -> p kc t", p=P))

            bq_sb = persist.tile([P, KC], F32)
            nc.sync.dma_start(bq_sb[:], bq_in.rearrange("(kc p) -> p kc", p=P))
            bk_sb = persist.tile([1, DIM], F32R)
            nc.sync.dma_start(bk_sb[:], bk_in[:])
            bv_sb = persist.tile([1, DIM], F32R)
            nc.sync.dma_start(bv_sb[:], bv_in[:])
            gamma_bc = persist.tile([P, DIM], F32)
            nc.sync.dma_start(gamma_bc[:], bcast_dram_row(gamma_in, P))
            beta_bc = persist.tile([P, DIM], F32)
            nc.sync.dma_start(beta_bc[:], bcast_dram_row(beta_in, P))
            eps_sb = persist.tile([P, 1], F32)
            nc.vector.memset(eps_sb[:], EPS_LN)

            # f32r constants must come from a rounding producer (copy), not memset
            ones_f32 = persist.tile([P, 1], F32)
            nc.vector.memset(ones_f32[:], 1.0)
            # fp32r matmul needs even free dims: ksum rhs is [1, 0] per row
            ones_col2_f32 = persist.tile([P, 2], F32)
            nc.vector.memset(ones_col2_f32[:], 0.0)
            nc.vector.memset(ones_col2_f32[:, 0:1], 1.0)
            ones_col2 = persist.tile([P, 2], F32R)  # ksum rhs
            nc.vector.tensor_copy(ones_col2[:], ones_col2_f32[:])
            ones_row = persist.tile([1, P], F32R)   # bias pre-MM lhsT
            nc.vector.tensor_copy(ones_row[:], ones_f32[0:1, 0:1].broadcast_to([1, P]))
            zeros_f32 = persist.tile([P, P], F32)
            nc.vector.memset(zeros_f32[:], 0.0)

            kv_send = persist.tile([P, NPAIR * HD + KC], F32)  # [128, 520]
            kv_red = persist.tile([P, NPAIR * HD + KC], F32)
            kvbd = persist.tile([P, NPAIR, P], F32R)   # block-diag kv pairs
            ksbd = persist.tile([P, NPAIR, 2], F32R)   # block-diag k_sum pairs

            # ---------------- Phase 1: k, v projections; kv & k_sum ---------
            # NOTE: multiple interleaved start=True accumulation chains that
            # share a PSUM bank wipe each other (each start zeroes the bank).
            # So kv/ksum use single-shot MMs per token tile, accumulated in
            # SBUF. ksum rides along as a ones-column: rhs = [v_pair | 1 0].
            KVW = P + 2  # 130, even (fp32r requires even free dims)
            kv_acc = persist.tile([P, NPAIR, KVW], F32)
            nc.vector.memset(kv_acc[:], 0.0)
            with (
                tc.tile_pool(name="wkv", bufs=1) as wkv,
                tc.tile_pool(name="kvtiles", bufs=3) as kvtiles,
                tc.tile_pool(name="p1psum", bufs=3, space="PSUM") as p1psum,
                tc.tile_pool(name="kvpsum", bufs=3, space="PSUM") as kvpsum,
            ):
                wkt_sb = wkv.tile([P, KC, DIM], F32R)
                nc.sync.dma_start(wkt_sb[:], wkt_in.rearrange("(kc p) o -> p kc o", p=P))
                wvt_sb = wkv.tile([P, KC, DIM], F32R)
                nc.sync.dma_start(wvt_sb[:], wvt_in.rearrange("(kc p) o -> p kc o", p=P))

                for i in range(TT1):
                    k_sb = kvtiles.tile([P, DIM], F32R, tag="k_sb")
                    v_sb = kvtiles.tile([P, NPAIR, KVW], F32R, tag="v_sb")
                    nc.vector.tensor_copy(
                        v_sb[:, :, P:],
                        ones_col2[:].rearrange("p (o t) -> p o t", o=1)
                        .broadcast_to([P, NPAIR, 2]))
                    for half in range(2):
                        oc = ts(half, F2)
                        ps = p1psum.tile([P, F2], F32, tag="proj")
                        nc.tensor.matmul(ps[:], ones_row[:], bk_sb[:, oc],
                                         start=True, stop=False)
                        for c in range(KC):
                            nc.tensor.matmul(
                                ps[:], xt_sb[:, c, ts(i, P)], wkt_sb[:, c, oc],
                                start=False, stop=(c == KC - 1))
                        nc.scalar.activation(k_sb[:, oc], ps[:], AF.Relu)
                        ps = p1psum.tile([P, F2], F32, tag="proj")
                        nc.tensor.matmul(ps[:], ones_row[:], bv_sb[:, oc],
                                         start=True, stop=False)
                        for c in range(KC):
                            nc.tensor.matmul(
                                ps[:], xt_sb[:, c, ts(i, P)], wvt_sb[:, c, oc],
                                start=False, stop=(c == KC - 1))
                        nc.scalar.activation(
                            v_sb[:, half * (NPAIR // 2):(half + 1) * (NPAIR // 2),
                                 0:P], ps[:].rearrange("p (n c) -> p n c", c=P),
                            AF.Copy)
                    if dbg and i == 0:
                        nc.sync.dma_start(dbg_outs["dbg_k"][:], k_sb[:].bitcast(F32))
                        nc.sync.dma_start(
                            dbg_outs["dbg_v"].rearrange("p (n c) -> p n c", c=P),
                            v_sb[:, :, 0:P].bitcast(F32))
                    for p in range(NPAIR):
                        ps_kv = kvpsum.tile([P, KVW], F32, tag="kv")
                        nc.tensor.matmul(ps_kv[:], k_sb[:, ts(p, P)],
                                         v_sb[:, p, :], start=True, stop=True)
                        nc.vector.tensor_add(kv_acc[:, p, :], kv_acc[:, p, :],
                                             ps_kv[:])

                # extract diagonal 64x64 blocks + k_sum into kv_send
                for p in range(NPAIR):
                    nc.vector.tensor_copy(kv_send[0:HD, ts(p, HD)],
                                          kv_acc[0:HD, p, 0:HD])
                    nc.vector.tensor_copy(kv_send[HD:P, ts(p, HD)],
                                          kv_acc[HD:P, p, HD:P])
                nc.vector.tensor_copy(
                    kv_send[:, NPAIR * HD:].rearrange("p (c one) -> p c one", one=1),
                    kv_acc[:, :, P:P + 1])

            # ---------------- Phase 2: q proj, ctx/denom, residual, LN ------
            # The AllReduce is emitted after the first q-projection tile so
            # the PE works on q while the collective is on the wire.
            with (
                tc.tile_pool(name="wq", bufs=1) as wq,
                tc.tile_pool(name="qt", bufs=2) as qtp,
                tc.tile_pool(name="qpsum", bufs=2, space="PSUM") as qpsum,
                tc.tile_pool(name="ctxpsum", bufs=2, space="PSUM") as ctxpsum,
                tc.tile_pool(name="denpsum", bufs=2, space="PSUM") as denpsum,
                tc.tile_pool(name="work", bufs=3) as work,
                tc.tile_pool(name="small", bufs=4) as small,
            ):
                wqt_sb = wq.tile([P, KC, DIM], F32R)
                nc.sync.dma_start(wqt_sb[:], wqt_in.rearrange("(kc p) o -> p kc o", p=P))

                def qproj(j):
                    # qT projection: out [och, tok] so ctx can contract channels
                    qt_sb = qtp.tile([P, KC, F2], F32R, tag="qt")
                    for co in range(KC):
                        ps = qpsum.tile([P, F2], F32, tag="qproj")
                        for ci in range(KC):
                            nc.tensor.matmul(
                                ps[:], wqt_sb[:, ci, ts(co, P)],
                                xt_sb[:, ci, ts(j, F2)],
                                start=(ci == 0), stop=(ci == KC - 1))
                        # relu(q + bq) fused into the psum eviction
                        nc.scalar.activation(qt_sb[:, co, :], ps[:], AF.Relu,
                                             bias=bq_sb[:, co:co + 1])
                    return qt_sb

                qt_tiles = {0: qproj(0)}

                # ---- AllReduce kv/k_sum across token-half pairs ----
                cc_in = dram.tile([P, NPAIR * HD + KC], F32)
                cc_out = dram.tile([P, NPAIR * HD + KC], F32)
                nc.sync.dma_start(cc_in[:], kv_send[:])
                nc.gpsimd.collective_compute(
                    "AllReduce", ALU.add,
                    replica_groups=[[0, 1], [2, 3], [4, 5], [6, 7]],
                    ins=[cc_in.opt()], outs=[cc_out.opt()])
                nc.sync.dma_start(kv_red[:], cc_out[:])
                if dbg:
                    nc.sync.dma_start(dbg_outs["dbg_kvred"][:], kv_red[:])

                # rebuild block-diagonal kv / k_sum operands (zero off-diag)
                for p in range(NPAIR):
                    nc.vector.tensor_copy(kvbd[:, p, :], zeros_f32[:])
                    nc.vector.tensor_copy(kvbd[0:HD, p, 0:HD], kv_red[0:HD, ts(p, HD)])
                    nc.vector.tensor_copy(kvbd[HD:P, p, HD:P], kv_red[HD:P, ts(p, HD)])
                    nc.vector.tensor_copy(ksbd[:, p, :], zeros_f32[:, 0:2])
                    nc.vector.tensor_copy(
                        ksbd[0:HD, p, 0:1],
                        kv_red[0:HD, NPAIR * HD + p:NPAIR * HD + p + 1])
                    nc.vector.tensor_copy(
                        ksbd[HD:P, p, 1:2],
                        kv_red[HD:P, NPAIR * HD + p:NPAIR * HD + p + 1])

                for j in range(TT2):
                    qt_sb = qt_tiles.pop(j)
                    if j + 1 < TT2:
                        qt_tiles[j + 1] = qproj(j + 1)

                    if dbg and j == 0:
                        nc.sync.dma_start(dbg_outs["dbg_q"][:], qt_sb[:].bitcast(F32))
                    for s in range(F2 // P):
                        t0 = j * F2 + s * P  # first token of this 128-row block
                        ctx_ps = ctxpsum.tile([P, DIM], F32, tag="ctx")
                        den_ps = denpsum.tile([P, H], F32, tag="den")
                        for p in range(NPAIR):
                            lq = qt_sb[:, p, ts(s, P)]
                            nc.tensor.matmul(ctx_ps[:, ts(p, P)], lq, kvbd[:, p, :],
                                             start=True, stop=True)
                            nc.tensor.matmul(den_ps[:, 2 * p:2 * p + 2], lq,
                                             ksbd[:, p, :], start=True, stop=True)
                        if dbg and j == 0 and s == 0:
                            dctx = work.tile([P, DIM], F32, tag="dctx")
                            nc.vector.tensor_copy(dctx[:], ctx_ps[:])
                            nc.sync.dma_start(dbg_outs["dbg_ctx"][:], dctx[:])
                            dden = small.tile([P, H], F32, tag="dden")
                            nc.vector.tensor_copy(dden[:], den_ps[:])
                            nc.sync.dma_start(dbg_outs["dbg_den"][:], dden[:])

                        rec = small.tile([P, H], F32, tag="rec")
                        nc.vector.tensor_scalar_max(rec[:], den_ps[:], EPS_DENOM)
                        nc.vector.reciprocal(rec[:], rec[:])

                        xn_t = work.tile([P, DIM], F32, tag="xn")
                        nc.sync.dma_start(xn_t[:], xn_in[t0:t0 + P, :])

                        y_t = work.tile([P, DIM], F32, tag="y")
                        nc.vector.tensor_tensor(
                            y_t[:].rearrange("p (h d) -> p h d", d=HD),
                            ctx_ps[:].rearrange("p (h d) -> p h d", d=HD),
                            rec[:].broadcast_to([P, H, HD]),
                            ALU.mult)
                        nc.vector.tensor_add(y_t[:], y_t[:], xn_t[:])
                        if dbg and j == 0 and s == 0:
                            nc.sync.dma_start(dbg_outs["dbg_y"][:], y_t[:])

                        # LayerNorm over channels (free dim)
                        stats = small.tile([P, 2, nc.vector.BN_STATS_DIM], F32,
                                           tag="stats")
                        mv = small.tile([P, nc.vector.BN_AGGR_DIM], F32, tag="mv")
                        yg = y_t[:].rearrange("p (g f) -> p g f", g=2)
                        for g in range(2):
                            nc.vector.bn_stats(stats[:, g, :], yg[:, g, :])
                        nc.vector.bn_aggr(mv[:], stats[:])
                        std = small.tile([P, 1], F32, tag="std")
                        nc.scalar.activation(std[:], mv[:, 1:2], AF.Sqrt,
                                             bias=eps_sb[:])
                        nc.vector.reciprocal(std[:], std[:])
                        nc.vector.tensor_scalar(y_t[:], y_t[:], mv[:, 0:1], std[:],
                                                op0=ALU.subtract, op1=ALU.mult)
                        nc.vector.tensor_mul(y_t[:], y_t[:], gamma_bc[:])
                        out_t = work.tile([P, DIM], F32, tag="out")
                        nc.gpsimd.tensor_add(out_t[:], y_t[:], beta_bc[:])
                        nc.sync.dma_start(yn_out[t0:t0 + P, :], out_t[:])

    nc.compile()
    return nc


_CACHE: dict = {}


def _get_nc():
    if "nc" not in _CACHE:
        _CACHE["nc"] = build()
    return _CACHE["nc"]


def make_in_maps(x, Wq, bq, Wk, bk, Wv, bv, gamma, beta):
    x = np.asarray(x, dtype=np.float32)
    f32 = lambda a: np.ascontiguousarray(np.asarray(a, dtype=np.float32))
    wqt = f32(np.asarray(Wq, np.float32).T)
    wkt = f32(np.asarray(Wk, np.float32).T)
    wvt = f32(np.asarray(Wv, np.float32).T)
    bq, bk, bv = f32(bq), f32(bk).reshape(1, DIM), f32(bv).reshape(1, DIM)
    gamma, beta = f32(gamma), f32(beta)
    in_maps = []
    for c in range(N_CORES):
        b, half = divmod(c, 2)
        xs = x[b, half * T:(half + 1) * T, :]
        in_maps.append({
            "xt": f32(xs.T), "xn": f32(xs),
            "wqt": wqt, "wkt": wkt, "wvt": wvt,
            "bq": bq, "bk": bk, "bv": bv,
            "gamma": gamma, "beta": beta,
        })
    return in_maps


def kernel(x, Wq, bq, Wk, bk, Wv, bv, gamma, beta):
    nc = _get_nc()
    in_maps = make_in_maps(x, Wq, bq, Wk, bk, Wv, bv, gamma, beta)
    res = run_bass_kernel_spmd(nc, in_maps, core_ids=list(range(N_CORES)))
    out = np.empty((B, NTOK, DIM), dtype=np.float32)
    for c in range(N_CORES):
        b, half = divmod(c, 2)
        out[b, half * T:(half + 1) * T, :] = res.results[c]["yn"]
    return out



# revision 2
# speedup vs baseline: 1.0438x; 1.0438x over previous
"""LinearAttention (relu feature map) + residual + LayerNorm on 8 TRN2 cores.

Reference (per batch b):
  q = relu(x @ Wq.T + bq); k = relu(x @ Wk.T + bk); v = x @ Wv.T + bv
  kv[h] = sum_n k[n,h,:] outer v[n,h,:];  k_sum[h] = sum_n k[n,h,:]
  denom = max(q . k_sum, 1e-6); ctx = q @ kv
  y = ctx/denom + x; out = LayerNorm(y) * gamma + beta

Sharding: core c handles (b = c//2, token half = c%2) -> T=2048 tokens.
kv/k_sum are partial sums over the core's tokens; a pairwise AllReduce
([0,1],[2,3],...) merges them. Everything else is core-local.

All matmuls run in bf16 (inputs quantized host-side / at psum eviction;
accumulation stays fp32 in PSUM). The v-bias is folded in after the
AllReduce via kv += k_sum (x) bv, so v-proj chains carry no bias MM.
k_sum rides along the kv matmul as an extra ones-column of v, and the
denominator rides along the ctx matmul as two extra ksum-columns of kv.
"""
import numpy as np
import ml_dtypes

import concourse.bass as bass
import concourse.tile as tile
from concourse import bacc, mybir
from concourse.bass_utils import run_bass_kernel_spmd
from concourse.bass import ts

B, NTOK, DIM, H, HD = 4, 4096, 1024, 16, 64
T = 2048          # tokens per core
P = 128           # partitions
KC = DIM // P     # 8 channel chunks
NPAIR = KC        # 8 head pairs (one per 128-channel chunk)
TT1 = T // P      # 16 token tiles in phase 1
F2 = 512          # phase-2 token tile (free dim)
TT2 = T // F2     # 4 phase-2 tiles
KVW = P + 2       # kv columns + [1,0] ksum ride-along
EPS_DENOM = 1e-6
EPS_LN = 1e-5
N_CORES = 8

F32 = mybir.dt.float32
BF16 = mybir.dt.bfloat16
AF = mybir.ActivationFunctionType
ALU = mybir.AluOpType
BF = ml_dtypes.bfloat16


def build(trace_sim: bool = False) -> "bacc.Bacc":
    nc = bacc.Bacc("TRN2", target_bir_lowering=False, debug=False,
                   num_devices=N_CORES)

    xt_in = nc.dram_tensor("xt", [DIM, T], BF16, kind="ExternalInput").ap()
    xn_in = nc.dram_tensor("xn", [T, DIM], F32, kind="ExternalInput").ap()
    wqt_in = nc.dram_tensor("wqt", [DIM, DIM], BF16, kind="ExternalInput").ap()
    wkt_in = nc.dram_tensor("wkt", [DIM, DIM], BF16, kind="ExternalInput").ap()
    wvt_in = nc.dram_tensor("wvt", [DIM, DIM], BF16, kind="ExternalInput").ap()
    bq_in = nc.dram_tensor("bq", [DIM], F32, kind="ExternalInput").ap()
    bk_in = nc.dram_tensor("bk", [1, DIM], BF16, kind="ExternalInput").ap()
    bvb_in = nc.dram_tensor("bvb", [P, F2], F32, kind="ExternalInput").ap()
    gamma_in = nc.dram_tensor("gamma", [DIM], F32, kind="ExternalInput").ap()
    beta_in = nc.dram_tensor("beta", [DIM], F32, kind="ExternalInput").ap()
    yn_out = nc.dram_tensor("yn", [T, DIM], F32, kind="ExternalOutput").ap()

    def bcast_dram_row(ap, n):
        # DRAM [D] -> [[0,n],[1,D]] so DMA replicates the row to n partitions
        return bass.AP(tensor=ap.tensor, offset=ap.offset,
                       ap=[[0, n]] + list(ap.ap))

    with tile.TileContext(nc, trace_sim=trace_sim) as tc:
        with (
            tc.tile_pool(name="persist", bufs=1) as persist,
            tc.tile_pool(name="dram", bufs=2, space="DRAM") as dram,
            tc.tile_pool(name="kvt", bufs=2) as kvt,
            tc.tile_pool(name="qtp", bufs=4) as qtp,
            tc.tile_pool(name="work", bufs=3) as work,
            tc.tile_pool(name="small", bufs=6) as small,
            tc.tile_pool(name="projp", bufs=2, space="PSUM") as projp,
            tc.tile_pool(name="kvp", bufs=2, space="PSUM") as kvp,
            tc.tile_pool(name="ctxp", bufs=2, space="PSUM") as ctxp,
        ):
            # ------------- persistent loads (issue order = priority) --------
            wkt_sb = persist.tile([P, KC, DIM], BF16)
            nc.sync.dma_start(wkt_sb[:], wkt_in.rearrange("(kc p) o -> p kc o", p=P))
            bk_sb = persist.tile([1, DIM], BF16)
            nc.sync.dma_start(bk_sb[:], bk_in[:])
            xt_sb = persist.tile([P, KC, T], BF16)
            xt_dram = xt_in.rearrange("(kc p) t -> p kc t", p=P)
            for ch in range(4):
                sl = slice(ch * (T // 4), (ch + 1) * (T // 4))
                nc.sync.dma_start(xt_sb[:, :, sl], xt_dram[:, :, sl])
            wvt_sb = persist.tile([P, KC, DIM], BF16)
            nc.sync.dma_start(wvt_sb[:], wvt_in.rearrange("(kc p) o -> p kc o", p=P))
            wqt_sb = persist.tile([P, KC, DIM], BF16)
            nc.sync.dma_start(wqt_sb[:], wqt_in.rearrange("(kc p) o -> p kc o", p=P))
            bq_sb = persist.tile([P, KC], F32)
            nc.sync.dma_start(bq_sb[:], bq_in.rearrange("(kc p) -> p kc", p=P))
            bvb_sb = persist.tile([P, F2], F32)
            nc.sync.dma_start(bvb_sb[:], bvb_in[:])
            gamma_bc = persist.tile([P, DIM], F32)
            nc.sync.dma_start(gamma_bc[:], bcast_dram_row(gamma_in, P))
            beta_bc = persist.tile([P, DIM], F32)
            nc.sync.dma_start(beta_bc[:], bcast_dram_row(beta_in, P))

            eps_sb = persist.tile([P, 1], F32)
            nc.vector.memset(eps_sb[:], EPS_LN)
            ones_row = persist.tile([1, P], BF16)
            nc.vector.memset(ones_row[:], 1.0)
            ones2 = persist.tile([P, 2], BF16)  # [1, 0] ksum ride-along cols
            nc.vector.memset(ones2[:, 0:1], 1.0)
            nc.vector.memset(ones2[:, 1:2], 0.0)

            kv_acc = persist.tile([P, NPAIR, KVW], F32)
            nc.vector.memset(kv_acc[:], 0.0)
            kv_send = persist.tile([P, NPAIR * HD + KC], F32)  # [128, 520]
            kv_red = persist.tile([P, NPAIR * HD + KC], F32)
            ksum_exp = persist.tile([P, F2], F32)
            kvkbd = persist.tile([P, NPAIR, KVW], BF16)  # blkdiag kv | ksum
            nc.vector.memset(kvkbd[:], 0.0)

            # ---------------- Phase 1: k, v projections; kv & k_sum ---------
            # Per token tile: 4 psum chains (k half0, v half0, k half1,
            # v half1), each contracting the 8 input-channel chunks with the
            # xt chunk stationary. The previous tile's 8 kv matmuls are
            # spread as 4 bursts of 2 between the chains so the DVE
            # accumulate (kv_acc +=) never stalls the PE.
            pend = None  # (k_sb, v_sb) of the previous tile

            def kv_burst(pk, pv, p0):
                for p in (p0, p0 + 1):
                    kps = kvp.tile([P, KVW], F32, tag="kv")
                    nc.tensor.matmul(kps[:], pk[:, ts(p, P)], pv[:, p, :],
                                     start=True, stop=True)
                    nc.vector.tensor_add(kv_acc[:, p, :], kv_acc[:, p, :],
                                         kps[:])

            for i in range(TT1):
                k_sb = kvt.tile([P, DIM], BF16, tag="k_sb")
                v_sb = kvt.tile([P, NPAIR, KVW], BF16, tag="v_sb")
                nc.vector.tensor_copy(
                    v_sb[:, :, P:],
                    ones2[:].rearrange("p (o t) -> p o t", o=1)
                    .broadcast_to([P, NPAIR, 2]))
                for idx, (kind, half) in enumerate(
                        (("k", 0), ("v", 0), ("k", 1), ("v", 1))):
                    oc = ts(half, F2)
                    ps = projp.tile([P, F2], F32, tag="proj")
                    if kind == "k":
                        nc.tensor.matmul(ps[:], ones_row[:], bk_sb[:, oc],
                                         start=True, stop=False)
                        for c in range(KC):
                            nc.tensor.matmul(
                                ps[:], xt_sb[:, c, ts(i, P)], wkt_sb[:, c, oc],
                                start=False, stop=(c == KC - 1))
                        nc.scalar.activation(k_sb[:, oc], ps[:], AF.Relu)
                    else:
                        for c in range(KC):
                            nc.tensor.matmul(
                                ps[:], xt_sb[:, c, ts(i, P)], wvt_sb[:, c, oc],
                                start=(c == 0), stop=(c == KC - 1))
                        nc.scalar.activation(
                            v_sb[:, half * (NPAIR // 2):(half + 1) * (NPAIR // 2),
                                 0:P], ps[:].rearrange("p (n c) -> p n c", c=P),
                            AF.Copy)
                    if pend is not None:
                        kv_burst(pend[0], pend[1], 2 * idx)
                pend = (k_sb, v_sb)
            for p0 in range(0, NPAIR, 2):
                kv_burst(pend[0], pend[1], p0)

            # pack diagonal 64x64 blocks + k_sum into kv_send (fp32)
            for p in range(NPAIR):
                nc.vector.tensor_copy(kv_send[0:HD, ts(p, HD)],
                                      kv_acc[0:HD, p, 0:HD])
                nc.vector.tensor_copy(kv_send[HD:P, ts(p, HD)],
                                      kv_acc[HD:P, p, HD:P])
            nc.vector.tensor_copy(
                kv_send[:, NPAIR * HD:].rearrange("p (c one) -> p c one", one=1),
                kv_acc[:, :, P:P + 1])
            cc_in = dram.tile([P, NPAIR * HD + KC], F32)
            cc_out = dram.tile([P, NPAIR * HD + KC], F32)
            nc.sync.dma_start(cc_in[:], kv_send[:])

            # ---------------- Phase 2: q proj, ctx/denom, residual, LN ------
            def qproj(j):
                # qT projection: out [och, tok] so ctx can contract channels
                qt_sb = qtp.tile([P, KC, F2], BF16, tag="qt")
                for co in range(KC):
                    ps = projp.tile([P, F2], F32, tag="proj")
                    for ci in range(KC):
                        nc.tensor.matmul(
                            ps[:], wqt_sb[:, ci, ts(co, P)],
                            xt_sb[:, ci, ts(j, F2)],
                            start=(ci == 0), stop=(ci == KC - 1))
                    # relu(q + bq) fused into the psum eviction
                    nc.scalar.activation(qt_sb[:, co, :], ps[:], AF.Relu,
                                         bias=bq_sb[:, co:co + 1])
                return qt_sb

            qts = [qproj(0)]

            # AllReduce kv/k_sum across token-half pairs; PE chews on q
            # projections while the collective is on the wire.
            nc.gpsimd.collective_compute(
                "AllReduce", ALU.add,
                replica_groups=[[0, 1], [2, 3], [4, 5], [6, 7]],
                ins=[cc_in.opt()], outs=[cc_out.opt()])

            qts.append(qproj(1))
            nc.sync.dma_start(kv_red[:], cc_out[:])

            # fold the v bias: kv += k_sum (x) bv  (exact: v only enters kv)
            nc.vector.tensor_copy(
                ksum_exp[:].rearrange("p (g c) -> p g c", g=NPAIR),
                kv_red[:, NPAIR * HD:].rearrange("p (g o) -> p g o", o=1)
                .broadcast_to([P, NPAIR, HD]))
            nc.vector.tensor_mul(ksum_exp[:], ksum_exp[:], bvb_sb[:])
            nc.vector.tensor_add(kv_red[:, 0:NPAIR * HD],
                                 kv_red[:, 0:NPAIR * HD], ksum_exp[:])
            # rebuild block-diagonal [kv | ksum] bf16 operands (pre-zeroed)
            for p in range(NPAIR):
                nc.vector.tensor_copy(kvkbd[0:HD, p, 0:HD],
                                      kv_red[0:HD, ts(p, HD)])
                nc.vector.tensor_copy(kvkbd[HD:P, p, HD:P],
                                      kv_red[HD:P, ts(p, HD)])
                nc.vector.tensor_copy(
                    kvkbd[0:HD, p, P:P + 1],
                    kv_red[0:HD, NPAIR * HD + p:NPAIR * HD + p + 1])
                nc.vector.tensor_copy(
                    kvkbd[HD:P, p, P + 1:P + 2],
                    kv_red[HD:P, NPAIR * HD + p:NPAIR * HD + p + 1])

            qts.append(qproj(2))

            def ctx_block(qt_sb, j, s):
                t0 = j * F2 + s * P
                xn_t = work.tile([P, DIM], F32, tag="xn")
                nc.sync.dma_start(xn_t[:], xn_in[t0:t0 + P, :])
                halves = []
                for hb in range(2):
                    cps = ctxp.tile([P, NPAIR // 2, KVW], F32, tag="ctx")
                    for pp in range(NPAIR // 2):
                        p = hb * (NPAIR // 2) + pp
                        nc.tensor.matmul(cps[:, pp, :],
                                         qt_sb[:, p, ts(s, P)],
                                         kvkbd[:, p, :], start=True, stop=True)
                    halves.append(cps)
                rec = small.tile([P, H], F32, tag="rec")
                for hb in range(2):
                    nc.vector.tensor_scalar_max(
                        rec[:, hb * NPAIR:(hb + 1) * NPAIR],
                        halves[hb][:, :, P:], EPS_DENOM)
                nc.vector.reciprocal(rec[:], rec[:])
                y_t = work.tile([P, DIM], F32, tag="y")
                for hb in range(2):
                    nc.vector.tensor_tensor(
                        y_t[:, ts(hb, F2)].rearrange(
                            "p (q h d) -> p q h d", q=NPAIR // 2, h=2),
                        halves[hb][:, :, 0:P].rearrange(
                            "p q (h d) -> p q h d", d=HD),
                        rec[:, ts(hb, NPAIR)].rearrange(
                            "p (q h) -> p q h", q=NPAIR // 2)
                        .broadcast_to([P, NPAIR // 2, 2, HD]),
                        ALU.mult)
                nc.gpsimd.tensor_add(y_t[:], y_t[:], xn_t[:])

                # LayerNorm over channels (free dim)
                stats = small.tile([P, 2, nc.vector.BN_STATS_DIM], F32,
                                   tag="stats")
                mv = small.tile([P, nc.vector.BN_AGGR_DIM], F32, tag="mv")
                yg = y_t[:].rearrange("p (g f) -> p g f", g=2)
                for g in range(2):
                    nc.vector.bn_stats(stats[:, g, :], yg[:, g, :])
                nc.vector.bn_aggr(mv[:], stats[:])
                std = small.tile([P, 1], F32, tag="std")
                nc.scalar.activation(std[:], mv[:, 1:2], AF.Sqrt,
                                     bias=eps_sb[:])
                nc.vector.reciprocal(std[:], std[:])
                nmi = small.tile([P, 1], F32, tag="nmi")
                nc.vector.tensor_scalar(nmi[:], mv[:, 0:1], std[:], -1.0,
                                        op0=ALU.mult, op1=ALU.mult)
                # (y - mu) * istd on the scalar engine: y*istd + (-mu*istd)
                nc.scalar.activation(y_t[:], y_t[:], AF.Identity,
                                     bias=nmi[:], scale=std[:])
                nc.vector.tensor_mul(y_t[:], y_t[:], gamma_bc[:])
                out_t = work.tile([P, DIM], F32, tag="out")
                nc.gpsimd.tensor_add(out_t[:], y_t[:], beta_bc[:])
                nc.sync.dma_start(yn_out[t0:t0 + P, :], out_t[:])

            for s in range(F2 // P):
                ctx_block(qts[0], 0, s)
            qts.append(qproj(3))
            for j in range(1, TT2):
                for s in range(F2 // P):
                    ctx_block(qts[j], j, s)

    nc.compile()
    return nc


_CACHE: dict = {}


def _get_nc():
    if "nc" not in _CACHE:
        _CACHE["nc"] = build()
    return _CACHE["nc"]


def make_in_maps(x, Wq, bq, Wk, bk, Wv, bv, gamma, beta):
    x = np.asarray(x, dtype=np.float32)
    f32 = lambda a: np.ascontiguousarray(np.asarray(a, dtype=np.float32))
    bf16 = lambda a: np.ascontiguousarray(np.asarray(a, dtype=np.float32)
                                          .astype(BF))
    wqt = bf16(np.asarray(Wq, np.float32).T)
    wkt = bf16(np.asarray(Wk, np.float32).T)
    wvt = bf16(np.asarray(Wv, np.float32).T)
    bq, bk = f32(bq), bf16(bk).reshape(1, DIM)
    # bvb[d_row, p*64+vd] = bv[(2p + (d_row>=64))*64 + vd]: the v-bias row
    # arranged to match kv_red's [d_row, (pair, vdim)] block layout.
    bv2 = np.asarray(bv, np.float32).reshape(NPAIR, 2, HD)
    bvb = np.empty((P, F2), np.float32)
    bvb[0:HD, :] = bv2[:, 0, :].reshape(1, F2)
    bvb[HD:P, :] = bv2[:, 1, :].reshape(1, F2)
    gamma, beta = f32(gamma), f32(beta)
    in_maps = []
    for c in range(N_CORES):
        b, half = divmod(c, 2)
        xs = x[b, half * T:(half + 1) * T, :]
        in_maps.append({
            "xt": bf16(xs.T), "xn": f32(xs),
            "wqt": wqt, "wkt": wkt, "wvt": wvt,
            "bq": bq, "bk": bk, "bvb": bvb,
            "gamma": gamma, "beta": beta,
        })
    return in_maps


def kernel(x, Wq, bq, Wk, bk, Wv, bv, gamma, beta):
    nc = _get_nc()
    in_maps = make_in_maps(x, Wq, bq, Wk, bk, Wv, bv, gamma, beta)
    res = run_bass_kernel_spmd(nc, in_maps, core_ids=list(range(N_CORES)))
    out = np.empty((B, NTOK, DIM), dtype=np.float32)
    for c in range(N_CORES):
        b, half = divmod(c, 2)
        out[b, half * T:(half + 1) * T, :] = res.results[c]["yn"]
    return out
